# revision 29
# baseline (speedup 1.0000x reference)
"""Sliding-window multi-head attention on 8 Trainium2 NeuronCores.

Sharding: 4 head-groups x 2 batches. Core (hg, b) computes q/k/v
projections for its 4 heads (d' = 256 dims, 2 slabs of 128) over its
batch's 2048 tokens, banded (window=256) attention for those heads,
and a partial output projection (its 256 rows of Wo^T). The host sums
the 4 head-group partials per batch and adds the bias. Batch is a
natural shard boundary (the window never crosses it): no halo, and the
per-core partial-output volume (the psum->sbuf copy burden that
saturates ACT/DVE) is half that of pure head-parallel sharding.

v4 (token-major PV, batch x head-group):
  - PV: u[128 q-tokens, 65] = ex2_slice.T @ [v|1] per q-block at full
    PE partition utilization; softmax normalize is a per-partition
    reciprocal[128,1] + scaled copy into the token-major stage.
  - PE transposes ([128,128] bf16, 128 cycles) restore d-major slabs
    for the output projection.
  - PSUM start=True zeroes a whole 2KB bank, so shared-bank groups
    (u accumulators, transpose slots in the u banks' spare columns)
    are memset then written with start=False only.
  - chunk0 runs e-major (4 open q or k banks) so the PE saturates
    while x pieces stream in; chunk1 is kind-major bg work (k, q, v
    order) consumed early inside the attention pass.
  - output drain: (512-token, m) units of two accumulating [128,512]
    matmuls (one per d-slab); m-pairs share one [128,1024] DMA that
    scatters to both m slabs via a dram-side dim permutation.
"""

import sys

sys.path.insert(0, "/opt/trn_rl_repo")

from contextlib import ExitStack

import numpy as np
from ml_dtypes import bfloat16

import concourse.bass as bass
import concourse.tile as tile
from concourse import bacc, mybir
from concourse.bass_utils import run_bass_kernel_spmd

F32 = mybir.dt.float32
F32R = mybir.dt.float32r
BF16 = mybir.dt.bfloat16
ACT_EXP = mybir.ActivationFunctionType.Exp
MUL = mybir.AluOpType.mult

N_CORES = 8
B, S, E = 2, 2048, 1024
H, D = 16, 64
NB = S // 128            # 16 key/query blocks in this core's batch
NH = 4                   # heads per core
WOFF = {"wq": 0, "wk": 2048, "wv": 4096, "wo": 6144}


class _Ctx:
    pass


def _emit(tc, io):
    import os
    nc = tc.nc
    with ExitStack() as ctx:
        const = ctx.enter_context(tc.tile_pool(name="const", bufs=1))
        big = ctx.enter_context(tc.tile_pool(name="big", bufs=1))
        xpool = ctx.enter_context(tc.tile_pool(name="xload", bufs=2))
        expool = ctx.enter_context(tc.tile_pool(name="expool", bufs=12))
        zpool = ctx.enter_context(tc.tile_pool(name="zpool", bufs=8))
        ostage = ctx.enter_context(tc.tile_pool(name="ostage", bufs=6))

        g = _Ctx()
        g.no_tp = bool(int(os.environ.get('NO_TP', '0')))

        wpack = const.tile([128, 8192], BF16, tag="wpack")
        mpack = const.tile([128, 384], BF16, tag="mpack")

        # lhsT blocks: kind in wq/wk/wv, e-chunk, d-slab (0/1)
        g.w = lambda kind, e, s: wpack[
            :, WOFF[kind] + 256 * e + 128 * s : WOFF[kind] + 256 * e + 128 * s + 128
        ]
        # wo lhsT: slab s (contraction rows), m slab of output dims
        g.wo = lambda s, m: wpack[
            :, WOFF["wo"] + 1024 * s + 128 * m : WOFF["wo"] + 1024 * s + 128 * m + 128
        ]
        g.maskL = mpack[:, 0:128]
        g.maskR = mpack[:, 128:256]
        g.ident = mpack[:, 256:384]

        # ---- persistent activation buffers -----------------------------
        # qT/kT: d-major, slab s at cols 2048*s. vA blocks are 260 wide:
        # [h0 d'(64)|1 | h1 |1 | h2 |1 | h3 |1]; each head's PV rhs
        # slice [.., 65] carries the softmax denominator column.
        g.qT = big.tile([128, 2 * S], BF16, tag="qT")
        g.kT = big.tile([128, 2 * S], BF16, tag="kT")
        g.vA = big.tile([128, NB * 260], BF16, tag="vA")
        g.aoTM = big.tile([128, NB * 256], BF16, tag="aoTM")  # token-major
        g.aoTd = big.tile([128, 2 * S], BF16, tag="aoTd")     # d-major slabs
        vA3 = g.vA[:, 0 : NB * 260].rearrange("p (blk c) -> p blk c", blk=NB)
        for h in range(NH):
            nc.gpsimd.memset(vA3[:, :, 65 * h + 64 : 65 * h + 65], 1.0)

        def _copy(sel, dst, src):
            # psum sources: only DVE/ACT may read PSUM
            if sel % 2:
                nc.scalar.copy(dst, src)
            else:
                nc.vector.tensor_copy(dst, src)

        def _vcopy(sel, tb, s, vps):
            # both heads of slab s in one strided copy (skips the ones
            # columns at +64 / +129 of the 130-wide half-block)
            dst = g.vA[:, 260 * tb + 130 * s : 260 * tb + 130 * s + 130]
            dst3 = dst.rearrange("p (h c) -> p h c", h=2)[:, :, 0:64]
            _copy(sel, dst3, vps[:].rearrange("p (h c) -> p h c", h=2))

        # ---- output-projection drain: unit = (c, m) --------------------
        # c indexes 512-token groups (0..3); every unit accumulates both
        # d-slabs into one [128,512] psum tile. m-pairs share one DMA.
        g.wo_ready = []
        g.cnt = [0] * 4
        g.drain_i = 0
        g.ost_half = {}

        def drain_wo(k, pools=None):
            pools = pools or [(psW, "w")]
            for _ in range(min(k, len(g.wo_ready))):
                c, m = g.wo_ready.pop(0)
                mp = m // 2
                if (c, mp) in g.ost_half:
                    ost = g.ost_half.pop((c, mp))
                else:
                    ost = ostage.tile([128, 1024], BF16, tag="ost", name="ost")
                    g.ost_half[(c, mp)] = ost
                pool, ptag = pools[g.drain_i % len(pools)]
                g.drain_i += 1
                wps = pool.tile([128, 512], F32, tag=ptag, name="wps")
                for s in range(2):
                    nc.tensor.matmul(
                        wps[:], g.wo(s, m),
                        g.aoTd[:, 2048 * s + 512 * c : 2048 * s + 512 * c + 512],
                        start=(s == 0), stop=(s == 1),
                    )
                half = m % 2
                _copy(c + m, ost[:, 512 * half : 512 * half + 512], wps[:])
                if half == 1:
                    nc.sync.dma_start(
                        io["outT"][2 * mp : 2 * mp + 2, :, 512 * c : 512 * c + 512]
                        .rearrange("m p x -> p m x"),
                        ost[:].rearrange("p (m x) -> p m x", m=2),
                    )

        # ---- projection loads (chunk n = tokens 1024n..+1024) ----------
        def load_piece(xt, n, e):
            nc.sync.dma_start(
                xt[:, 1024 * e : 1024 * e + 1024],
                io["xT"][e, :, 1024 * n : 1024 * n + 1024],
            )

        # ---- e-major projection for chunk0 (startup; 4+2 psum banks) --
        def proj_chunk_emajor(n, xt, qk, v0):
            t0 = 1024 * n
            for kind, cp in (("wq", 0), ("wk", 1)):
                ph = [qk.tile([128, 512], F32, tag=f"qk{i}", name="ph")
                      for i in range(4)]
                for e in range(8):
                    for s in range(2):
                        for half in range(2):
                            nc.tensor.matmul(
                                ph[2 * s + half][:], g.w(kind, e, s),
                                xt[:, 1024 * e + 512 * half : 1024 * e + 512 * half + 512],
                                start=(e == 0), stop=(e == 7),
                            )
                dst = g.qT if kind == "wq" else g.kT
                for s in range(2):
                    for half in range(2):
                        _copy(cp + s + half,
                              dst[:, 2048 * s + t0 + 512 * half :
                                  2048 * s + t0 + 512 * half + 512],
                              ph[2 * s + half][:])
            for i in range(8):
                tb = 8 * n + i
                for s in range(2):
                    vps = v0.tile([128, 128], F32, tag="v", name="vps")
                    for e in range(8):
                        nc.tensor.matmul(
                            vps[:], xt[:, 1024 * e + 128 * i : 1024 * e + 128 * i + 128],
                            g.w("wv", e, s),
                            start=(e == 0), stop=(e == 7),
                        )
                    _vcopy(i + s, tb, s, vps)

        # ---- kind-major projection chunk (bg inside the pass) ----------
        # emission order k, q, v: scores need chunk1 kT from j=8 and its
        # qT from j=7; PV needs vA[tb] at j=tb.
        def chunk_steps(n, xt):
            t0 = 1024 * n
            for kind in ("wk", "wq"):
                for s in range(2):
                    for half in range(2):
                        ps = psMix.tile([128, 512], F32, tag="p")
                        for e in range(8):
                            nc.tensor.matmul(
                                ps[:], g.w(kind, e, s),
                                xt[:, 1024 * e + 512 * half : 1024 * e + 512 * half + 512],
                                start=(e == 0), stop=(e == 7),
                            )
                        dst = g.qT if kind == "wq" else g.kT
                        _copy(s + half + (1 if kind == "wq" else 0),
                              dst[:, 2048 * s + t0 + 512 * half :
                                  2048 * s + t0 + 512 * half + 512],
                              ps[:])
                        yield
            for i in range(8):
                tb = 8 * n + i
                for s in range(2):
                    vps = psMix.tile([128, 128], F32, tag="p")
                    for e in range(8):
                        nc.tensor.matmul(
                            vps[:], xt[:, 1024 * e + 128 * i : 1024 * e + 128 * i + 128],
                            g.w("wv", e, s),
                            start=(e == 0), stop=(e == 7),
                        )
                    _vcopy(i + s, tb, s, vps)
                    yield

        # ---- attention pass: 16 j blocks, 4 heads interleaved ----------
        _ub = None  # set after psum pool allocation below

        def finish(qb, h):
            u = g.uslot(qb, h)
            rz = zpool.tile([128, 1], F32R, tag="rz")
            with nc.allow_low_precision(reason="f32r is fp32-width"):
                nc.vector.reciprocal(rz[:], u[:, 64:65])
            dst = g.aoTM[:, 256 * qb + 64 * h : 256 * qb + 64 * h + 64]
            nc.vector.tensor_scalar(dst, u[:, 0:64], rz[:].bitcast(F32), None, MUL)
            if h == NH - 1 and not getattr(g, 'no_tp', False):
                for s in range(2):
                    tv = g.tslot(s)
                    nc.tensor.matmul(
                        tv, g.aoTM[:, 256 * qb + 128 * s : 256 * qb + 128 * s + 128],
                        g.ident,
                        is_transpose=True, start=True, stop=True,
                        skip_group_check=True,
                    )
                    nc.vector.tensor_copy(
                        g.aoTd[:, 2048 * s + 128 * qb : 2048 * s + 128 * qb + 128], tv)
                c = qb // 4
                g.cnt[c] += 1
                if g.cnt[c] == 4:
                    g.wo_ready.extend((c, m) for m in range(8))

        def attn_pass(bg=None):
            def bg_step(j):
                if bg is not None:
                    next(bg, None)
                    if j < 12:
                        next(bg, None)
                k = 4 if len(g.wo_ready) >= 12 else 2
                drain_wo(k, pools=[(psW, "w")])

            def emit_sxm(j, h):
                q0 = 128 * max(j - 1, 0)
                W = min(128 * (j + 2), S) - q0
                s = h // 2
                p0 = 64 * (h % 2)
                with tc.high_priority(offset=60):
                    sT = psS.tile([128, 384], F32, tag="s")
                    nc.tensor.matmul(
                        sT[:, 0:W],
                        g.kT[p0 : p0 + 64, 2048 * s + 128 * j : 2048 * s + 128 * j + 128],
                        g.qT[p0 : p0 + 64, 2048 * s + q0 : 2048 * s + q0 + W],
                        start=True, stop=True,
                    )
                    ex2 = expool.tile([128, 384], BF16, tag="ex")
                    nc.scalar.activation(ex2[:, 0:W], sT[:, 0:W], ACT_EXP)
                    if j > 0:
                        nc.gpsimd.tensor_tensor(ex2[:, 0:128], ex2[:, 0:128],
                                                g.maskL, MUL)
                    if j < NB - 1:
                        nc.gpsimd.tensor_tensor(ex2[:, W - 128 : W], ex2[:, W - 128 : W],
                                                g.maskR, MUL)
                return ex2

            for bank in (ubankA, ubankB):          # qb 0,1 slots (all heads)
                nc.vector.memset(bank[:, 0:260], 0.0)
            ex2 = [emit_sxm(0, h) for h in range(NH)]
            for j in range(NB):
                ex2_next = [emit_sxm(j + 1, h) for h in range(NH)] if j + 1 < NB else None
                bg_step(j)
                q0b = max(j - 1, 0)
                for h in range(NH):
                    for qb in range(q0b, min(j + 2, NB)):
                        col0 = 128 * (qb - q0b)
                        nc.tensor.matmul(
                            g.uslot(qb, h),
                            ex2[h][:, col0 : col0 + 128],
                            g.vA[:, 260 * j + 65 * h : 260 * j + 65 * h + 65],
                            start=False,
                            stop=(j == min(qb + 1, NB - 1)),
                            skip_group_check=True,
                        )
                for qb in ([j - 1] if j > 0 else []) + ([NB - 1] if j == NB - 1 else []):
                    for h in range(NH):
                        finish(qb, h)
                if j + 2 < NB:
                    # zero qb=j+2's slot pairs (reuse qb=j-1's, read above)
                    o = 130 * ((j + 2) % 3)
                    nc.vector.memset(ubankA[:, o : o + 130], 0.0)
                    nc.vector.memset(ubankB[:, o : o + 130], 0.0)
                ex2 = ex2_next
            if bg is not None:
                for _ in bg:
                    pass

        # ---- schedule ---------------------------------------------------
        # DMA order: wq, x0p0-3, wk, x0p4-7, wv, x1, mpack, wo
        xt0 = xpool.tile([128, 8192], BF16, tag="xtc")
        xt1 = xpool.tile([128, 8192], BF16, tag="xtc")
        nc.sync.dma_start(wpack[:, 0:256], io["wpack"][:, 0:256])
        load_piece(xt0, 0, 0)
        nc.sync.dma_start(wpack[:, 256:2048], io["wpack"][:, 256:2048])
        for e in range(1, 4):
            load_piece(xt0, 0, e)
        nc.sync.dma_start(wpack[:, 2048:4096], io["wpack"][:, 2048:4096])
        for e in range(4, 8):
            load_piece(xt0, 0, e)
        nc.sync.dma_start(wpack[:, 4096:6144], io["wpack"][:, 4096:6144])
        for e in range(8):
            load_piece(xt1, 1, e)
        nc.sync.dma_start(mpack[:], io["mpack"][:])
        nc.sync.dma_start(wpack[:, 6144:8192], io["wpack"][:, 6144:8192])

        with tc.tile_pool(name="qk0", bufs=1, space="PSUM") as qk, \
             tc.tile_pool(name="v0", bufs=2, space="PSUM") as v0:
            proj_chunk_emajor(0, xt0, qk, v0)

        psMix = ctx.enter_context(tc.tile_pool(name="psMix", bufs=2, space="PSUM"))
        psS = ctx.enter_context(tc.tile_pool(name="psS", bufs=1, space="PSUM"))
        tpool = ctx.enter_context(tc.tile_pool(name="tpool", bufs=1, space="PSUM"))
        psW = ctx.enter_context(tc.tile_pool(name="psW", bufs=2, space="PSUM"))
        uA = ctx.enter_context(tc.tile_pool(name="uA", bufs=1, space="PSUM"))
        uB = ctx.enter_context(tc.tile_pool(name="uB", bufs=1, space="PSUM"))
        # u slots: 3 rotating qb slots x 2 heads per bank (cols 0:390);
        # transpose slots live in the banks' spare cols 448:512 as bf16.
        # start=True must never touch these banks: memset + start=False.
        ubankA = uA.tile([128, 512], F32, tag="uA")
        ubankB = uB.tile([128, 512], F32, tag="uB")
        _ub = [ubankA, ubankB]
        g.uslot = lambda qb, h: _ub[h // 2][
            :, 130 * (qb % 3) + 65 * (h % 2) : 130 * (qb % 3) + 65 * (h % 2) + 65
        ]
        tbank = tpool.tile([128, 256], BF16, tag="tbank")
        g.tslot = lambda s: tbank[:, 128 * (s % 2) : 128 * (s % 2) + 128]

        attn_pass(bg=chunk_steps(1, xt1))
        drain_wo(64, pools=[(psW, "w"), (psMix, "p")])

        if "dbg" in io:
            nc.sync.dma_start(io["dbg"][:, 0 : 2 * S], g.qT[:])
            nc.sync.dma_start(io["dbg"][:, 2 * S : 4 * S], g.kT[:])
            nc.sync.dma_start(io["dbg"][:, 4 * S : 4 * S + NB * 256], g.aoTM[:])
            if not g.no_tp:
                nc.sync.dma_start(io["dbg"][:, 6 * S : 8 * S], g.aoTd[:])
            nc.sync.dma_start(io["dbg"][:, 8 * S : 8 * S + NB * 260], g.vA[:])


def build_program(dbg=False):
    nc = bacc.Bacc("TRN2", target_bir_lowering=False, debug=False, num_devices=N_CORES)
    io = {}

    def inp(name, shape):
        io[name] = nc.dram_tensor(name, shape, BF16, kind="ExternalInput").ap()

    inp("xT", [8, 128, S])
    inp("wpack", [128, 8192])
    inp("mpack", [128, 384])
    io["outT"] = nc.dram_tensor("outT", [8, 128, S], BF16, kind="ExternalOutput").ap()
    if dbg:
        io["dbg"] = nc.dram_tensor("dbg", [128, 8 * S + NB * 260], BF16,
                                   kind="ExternalOutput").ap()

    with tile.TileContext(nc) as tc:
        _emit(tc, io)
    nc.compile()
    return nc


def _host_inputs(x, Wq, Wk, Wv, Wo):
    """Per-core input maps. Core index = 2*hg + b."""
    r = np.arange(128)[:, None]
    c = np.arange(128)[None, :]
    m_left = (r <= c).astype(np.float32)
    m_right = (r >= c).astype(np.float32)
    mpack = np.concatenate(
        [m_left, m_right, np.eye(128, dtype=np.float32)], axis=1
    ).astype(bfloat16)

    scale = 1.0 / np.sqrt(D)
    xTb = []
    for b in range(B):
        xf = np.ascontiguousarray(x[b].T).astype(bfloat16)   # [1024, 2048]
        xTb.append(xf.reshape(8, 128, S))

    in_maps = []
    for core in range(N_CORES):
        hg, b = core // 2, core % 2
        rows = slice(256 * hg, 256 * hg + 256)
        wq = np.ascontiguousarray((Wq[rows, :] * scale).T)   # [1024 e, 256 d']
        wk = np.ascontiguousarray(Wk[rows, :].T)
        wv = np.ascontiguousarray(Wv[rows, :].T)
        wqc = wq.reshape(8, 128, 256)
        wkc = wk.reshape(8, 128, 256)
        wvc = wv.reshape(8, 128, 256)
        wpk = np.zeros((128, 8192), dtype=np.float32)
        for e in range(8):
            wpk[:, 0 + 256 * e : 256 * e + 256] = wqc[e]
            wpk[:, 2048 + 256 * e : 2304 + 256 * e] = wkc[e]
            wpk[:, 4096 + 256 * e : 4352 + 256 * e] = wvc[e]
        for s in range(2):
            wos = Wo[:, 256 * hg + 128 * s : 256 * hg + 128 * s + 128].T  # [128, 1024]
            for m in range(8):
                wpk[:, 6144 + 1024 * s + 128 * m : 6144 + 1024 * s + 128 * m + 128] = \
                    wos[:, 128 * m : 128 * m + 128]
        in_maps.append(
            {"xT": xTb[b], "wpack": wpk.astype(bfloat16), "mpack": mpack}
        )
    return in_maps


_NC_CACHE = None


def kernel(x, Wq, Wk, Wv, Wo, bo):
    global _NC_CACHE
    x = np.asarray(x, dtype=np.float32)
    Wq = np.asarray(Wq, dtype=np.float32)
    Wk = np.asarray(Wk, dtype=np.float32)
    Wv = np.asarray(Wv, dtype=np.float32)
    Wo = np.asarray(Wo, dtype=np.float32)
    bo = np.asarray(bo, dtype=np.float32)

    if _NC_CACHE is None:
        _NC_CACHE = build_program()
    nc = _NC_CACHE

    in_maps = _host_inputs(x, Wq, Wk, Wv, Wo)
    res = run_bass_kernel_spmd(nc, in_maps, core_ids=list(range(N_CORES)))

    out = np.zeros((B, S, E), dtype=np.float32)
    for core in range(N_CORES):
        hg, b = core // 2, core % 2
        out[b] += res.results[core]["outT"].astype(np.float32).reshape(E, S).T
    return np.ascontiguousarray(out + bo[None, None, :])


# revision 30
# speedup vs baseline: 1.1304x; 1.1304x over previous
"""Sliding-window multi-head attention on 8 Trainium2 NeuronCores.

Sharding: 4 head-groups x 2 batches. Core (hg, b) computes q/k/v
projections for its 4 heads (d' = 256 dims, 2 slabs of 128) over its
batch's 2048 tokens, banded (window=256) attention for those heads,
and a partial output projection (its 256 rows of Wo^T). The host sums
the 4 head-group partials per batch and adds the bias. Batch is a
natural shard boundary (the window never crosses it): no halo, and the
per-core partial-output volume (the psum->sbuf copy burden that
saturates ACT/DVE) is half that of pure head-parallel sharding.

v4 (token-major PV, batch x head-group):
  - PV: u[128 q-tokens, 65] = ex2_slice.T @ [v|1] per q-block at full
    PE partition utilization; softmax normalize is a per-partition
    reciprocal[128,1] + scaled copy into the token-major stage.
  - PE transposes ([128,128] bf16, 128 cycles) restore d-major slabs
    for the output projection.
  - PSUM start=True zeroes a whole 2KB bank, so shared-bank groups
    (u accumulators, transpose slots in the u banks' spare columns)
    are memset then written with start=False only.
  - chunk0 runs e-major (4 open q or k banks) so the PE saturates
    while x pieces stream in; chunk1 is kind-major bg work (k, q, v
    order) consumed early inside the attention pass.
  - output drain: (512-token, m) units of two accumulating [128,512]
    matmuls (one per d-slab); m-pairs share one [128,1024] DMA that
    scatters to both m slabs via a dram-side dim permutation.
"""

import sys

sys.path.insert(0, "/opt/trn_rl_repo")

from contextlib import ExitStack

import numpy as np
from ml_dtypes import bfloat16

import concourse.bass as bass
import concourse.tile as tile
from concourse import bacc, mybir
from concourse.bass_utils import run_bass_kernel_spmd

F32 = mybir.dt.float32
F32R = mybir.dt.float32r
BF16 = mybir.dt.bfloat16
ACT_EXP = mybir.ActivationFunctionType.Exp
MUL = mybir.AluOpType.mult

N_CORES = 8
B, S, E = 2, 2048, 1024
H, D = 16, 64
NB = S // 128            # 16 key/query blocks in this core's batch
NH = 4                   # heads per core
WOFF = {"wq": 0, "wk": 2048, "wv": 4096, "wo": 6144}


class _Ctx:
    pass


def _emit(tc, io):
    import os
    nc = tc.nc
    with ExitStack() as ctx:
        const = ctx.enter_context(tc.tile_pool(name="const", bufs=1))
        big = ctx.enter_context(tc.tile_pool(name="big", bufs=1))
        xpool = ctx.enter_context(tc.tile_pool(name="xload", bufs=2))
        expool = ctx.enter_context(tc.tile_pool(name="expool", bufs=12))
        zpool = ctx.enter_context(tc.tile_pool(name="zpool", bufs=8))
        ostage = ctx.enter_context(tc.tile_pool(name="ostage", bufs=6))

        g = _Ctx()
        g.no_tp = bool(int(os.environ.get('NO_TP', '0')))

        wpack = const.tile([128, 8192], BF16, tag="wpack")
        mpack = const.tile([128, 384], BF16, tag="mpack")

        # lhsT blocks: kind in wq/wk/wv, e-chunk, d-slab (0/1)
        g.w = lambda kind, e, s: wpack[
            :, WOFF[kind] + 256 * e + 128 * s : WOFF[kind] + 256 * e + 128 * s + 128
        ]
        # wo lhsT: slab s (contraction rows), m slab of output dims
        g.wo = lambda s, m: wpack[
            :, WOFF["wo"] + 1024 * s + 128 * m : WOFF["wo"] + 1024 * s + 128 * m + 128
        ]
        g.maskL = mpack[:, 0:128]
        g.maskR = mpack[:, 128:256]
        g.ident = mpack[:, 256:384]

        # ---- persistent activation buffers -----------------------------
        # qT/kT: d-major, slab s at cols 2048*s. vA blocks are 260 wide:
        # [h0 d'(64)|1 | h1 |1 | h2 |1 | h3 |1]; each head's PV rhs
        # slice [.., 65] carries the softmax denominator column.
        g.qT = big.tile([128, 2 * S], BF16, tag="qT")
        g.kT = big.tile([128, 2 * S], BF16, tag="kT")
        g.vA = big.tile([128, NB * 260], BF16, tag="vA")
        g.aoTM = big.tile([128, NB * 256], BF16, tag="aoTM")  # token-major
        g.aoTd = big.tile([128, 2 * S], BF16, tag="aoTd")     # d-major slabs
        vA3 = g.vA[:, 0 : NB * 260].rearrange("p (blk c) -> p blk c", blk=NB)
        for h in range(NH):
            nc.gpsimd.memset(vA3[:, :, 65 * h + 64 : 65 * h + 65], 1.0)

        def _copy(sel, dst, src):
            # psum sources: only DVE/ACT may read PSUM
            if sel % 2:
                nc.scalar.copy(dst, src)
            else:
                nc.vector.tensor_copy(dst, src)

        def _vcopy(sel, tb, s, vps):
            # both heads of slab s in one strided copy (skips the ones
            # columns at +64 / +129 of the 130-wide half-block)
            dst = g.vA[:, 260 * tb + 130 * s : 260 * tb + 130 * s + 130]
            dst3 = dst.rearrange("p (h c) -> p h c", h=2)[:, :, 0:64]
            _copy(sel, dst3, vps[:].rearrange("p (h c) -> p h c", h=2))

        # ---- output-projection drain: unit = (c, m) --------------------
        # c indexes 512-token groups (0..3); every unit accumulates both
        # d-slabs into one [128,512] psum tile. m-pairs share one DMA.
        g.wo_ready = []
        g.cnt = [0] * 4
        g.drain_i = 0
        g.ost_half = {}

        def drain_wo(k, pools=None):
            pools = pools or [(psMix, "p")]
            for _ in range(min(k, len(g.wo_ready))):
                c, m = g.wo_ready.pop(0)
                mp = m // 2
                if (c, mp) in g.ost_half:
                    ost = g.ost_half.pop((c, mp))
                else:
                    ost = ostage.tile([128, 1024], BF16, tag="ost", name="ost")
                    g.ost_half[(c, mp)] = ost
                pool, ptag = pools[g.drain_i % len(pools)]
                g.drain_i += 1
                wps = pool.tile([128, 512], F32, tag=ptag, name="wps")
                for s in range(2):
                    nc.tensor.matmul(
                        wps[:], g.wo(s, m),
                        g.aoTd[:, 2048 * s + 512 * c : 2048 * s + 512 * c + 512],
                        start=(s == 0), stop=(s == 1),
                    )
                half = m % 2
                _copy(c + m, ost[:, 512 * half : 512 * half + 512], wps[:])
                if half == 1:
                    nc.sync.dma_start(
                        io["outT"][2 * mp : 2 * mp + 2, :, 512 * c : 512 * c + 512]
                        .rearrange("m p x -> p m x"),
                        ost[:].rearrange("p (m x) -> p m x", m=2),
                    )

        # ---- projection loads (chunk n = tokens 1024n..+1024) ----------
        def load_piece(xt, n, e):
            nc.sync.dma_start(
                xt[:, 1024 * e : 1024 * e + 1024],
                io["xT"][e, :, 1024 * n : 1024 * n + 1024],
            )

        # ---- e-major projection for chunk0 (startup; 4+2 psum banks) --
        def proj_chunk_emajor(n, xt, qk, v0):
            t0 = 1024 * n
            for kind, cp in (("wq", 0), ("wk", 1)):
                ph = [qk.tile([128, 512], F32, tag=f"qk{i}", name="ph")
                      for i in range(4)]
                for e in range(8):
                    for s in range(2):
                        for half in range(2):
                            nc.tensor.matmul(
                                ph[2 * s + half][:], g.w(kind, e, s),
                                xt[:, 1024 * e + 512 * half : 1024 * e + 512 * half + 512],
                                start=(e == 0), stop=(e == 7),
                            )
                dst = g.qT if kind == "wq" else g.kT
                for s in range(2):
                    for half in range(2):
                        _copy(cp + s + half,
                              dst[:, 2048 * s + t0 + 512 * half :
                                  2048 * s + t0 + 512 * half + 512],
                              ph[2 * s + half][:])
            for i in range(8):
                tb = 8 * n + i
                for s in range(2):
                    vps = v0.tile([128, 128], F32, tag="v", name="vps")
                    for e in range(8):
                        nc.tensor.matmul(
                            vps[:], xt[:, 1024 * e + 128 * i : 1024 * e + 128 * i + 128],
                            g.w("wv", e, s),
                            start=(e == 0), stop=(e == 7),
                        )
                    _vcopy(i + s, tb, s, vps)

        # ---- kind-major projection chunk (bg inside the pass) ----------
        # emission order k, q, v: scores need chunk1 kT from j=8 and its
        # qT from j=7; PV needs vA[tb] at j=tb.
        def chunk_steps(n, xt):
            t0 = 1024 * n
            for kind in ("wk", "wq"):
                for s in range(2):
                    for half in range(2):
                        ps = psMix.tile([128, 512], F32, tag="p")
                        for e in range(8):
                            nc.tensor.matmul(
                                ps[:], g.w(kind, e, s),
                                xt[:, 1024 * e + 512 * half : 1024 * e + 512 * half + 512],
                                start=(e == 0), stop=(e == 7),
                            )
                        dst = g.qT if kind == "wq" else g.kT
                        _copy(s + half + (1 if kind == "wq" else 0),
                              dst[:, 2048 * s + t0 + 512 * half :
                                  2048 * s + t0 + 512 * half + 512],
                              ps[:])
                        yield
            for i in range(8):
                tb = 8 * n + i
                for s in range(2):
                    vps = psMix.tile([128, 128], F32, tag="p")
                    for e in range(8):
                        nc.tensor.matmul(
                            vps[:], xt[:, 1024 * e + 128 * i : 1024 * e + 128 * i + 128],
                            g.w("wv", e, s),
                            start=(e == 0), stop=(e == 7),
                        )
                    _vcopy(i + s, tb, s, vps)
                    yield

        # ---- attention pass: 16 j blocks, 4 heads interleaved ----------
        _ub = None  # set after psum pool allocation below

        def finish(qb, h):
            u = g.uslot(qb, h)
            rz = zpool.tile([128, 1], F32R, tag="rz")
            with nc.allow_low_precision(reason="f32r is fp32-width"):
                nc.vector.reciprocal(rz[:], u[:, 64:65])
            dst = g.aoTM[:, 256 * qb + 64 * h : 256 * qb + 64 * h + 64]
            nc.vector.tensor_scalar(dst, u[:, 0:64], rz[:].bitcast(F32), None, MUL)
            if h == NH - 1 and not getattr(g, 'no_tp', False):
                for s in range(2):
                    tv = g.tslot(s)
                    nc.tensor.matmul(
                        tv, g.aoTM[:, 256 * qb + 128 * s : 256 * qb + 128 * s + 128],
                        g.ident,
                        is_transpose=True, start=True, stop=True,
                        skip_group_check=True,
                    )
                    nc.vector.tensor_copy(
                        g.aoTd[:, 2048 * s + 128 * qb : 2048 * s + 128 * qb + 128], tv)
                c = qb // 4
                g.cnt[c] += 1
                if g.cnt[c] == 4:
                    g.wo_ready.extend((c, m) for m in range(8))

        def attn_pass(bg=None):
            def bg_step(j):
                if bg is not None:
                    next(bg, None)
                    if j < 12:
                        next(bg, None)
                k = 4 if len(g.wo_ready) >= 12 else 2
                drain_wo(k, pools=[(psMix, "p")])

            def emit_sxm(j, h):
                q0 = 128 * max(j - 1, 0)
                W = min(128 * (j + 2), S) - q0
                s = h // 2
                p0 = 64 * (h % 2)
                with tc.high_priority(offset=60):
                    sT = psS.tile([128, 384], F32, tag="s")
                    nc.tensor.matmul(
                        sT[:, 0:W],
                        g.kT[p0 : p0 + 64, 2048 * s + 128 * j : 2048 * s + 128 * j + 128],
                        g.qT[p0 : p0 + 64, 2048 * s + q0 : 2048 * s + q0 + W],
                        start=True, stop=True,
                    )
                    ex2 = expool.tile([128, 384], BF16, tag="ex")
                    nc.scalar.activation(ex2[:, 0:W], sT[:, 0:W], ACT_EXP)
                    if j > 0:
                        nc.gpsimd.tensor_tensor(ex2[:, 0:128], ex2[:, 0:128],
                                                g.maskL, MUL)
                    if j < NB - 1:
                        nc.gpsimd.tensor_tensor(ex2[:, W - 128 : W], ex2[:, W - 128 : W],
                                                g.maskR, MUL)
                return ex2

            for bank in (ubankA, ubankB):          # qb 0,1 slots (all heads)
                nc.vector.memset(bank[:, 0:260], 0.0)
            ex2 = [emit_sxm(0, h) for h in range(NH)]
            for j in range(NB):
                ex2_next = [emit_sxm(j + 1, h) for h in range(NH)] if j + 1 < NB else None
                bg_step(j)
                q0b = max(j - 1, 0)
                for h in range(NH):
                    for qb in range(q0b, min(j + 2, NB)):
                        col0 = 128 * (qb - q0b)
                        nc.tensor.matmul(
                            g.uslot(qb, h),
                            ex2[h][:, col0 : col0 + 128],
                            g.vA[:, 260 * j + 65 * h : 260 * j + 65 * h + 65],
                            start=False,
                            stop=(j == min(qb + 1, NB - 1)),
                            skip_group_check=True,
                        )
                for qb in ([j - 1] if j > 0 else []) + ([NB - 1] if j == NB - 1 else []):
                    for h in range(NH):
                        finish(qb, h)
                if j + 2 < NB:
                    # zero qb=j+2's slot pairs (reuse qb=j-1's, read above)
                    o = 130 * ((j + 2) % 3)
                    nc.vector.memset(ubankA[:, o : o + 130], 0.0)
                    nc.vector.memset(ubankB[:, o : o + 130], 0.0)
                ex2 = ex2_next
            if bg is not None:
                for _ in bg:
                    pass

        # ---- schedule ---------------------------------------------------
        # DMA order: wq, x0p0-3, wk, x0p4-7, wv, x1, mpack, wo
        xt0 = xpool.tile([128, 8192], BF16, tag="xtc")
        xt1 = xpool.tile([128, 8192], BF16, tag="xtc")
        nc.sync.dma_start(wpack[:, 0:256], io["wpack"][:, 0:256])
        load_piece(xt0, 0, 0)
        nc.sync.dma_start(wpack[:, 256:2048], io["wpack"][:, 256:2048])
        for e in range(1, 4):
            load_piece(xt0, 0, e)
        nc.sync.dma_start(wpack[:, 2048:4096], io["wpack"][:, 2048:4096])
        for e in range(4, 8):
            load_piece(xt0, 0, e)
        nc.sync.dma_start(wpack[:, 4096:6144], io["wpack"][:, 4096:6144])
        for e in range(8):
            load_piece(xt1, 1, e)
        nc.sync.dma_start(mpack[:], io["mpack"][:])
        nc.sync.dma_start(wpack[:, 6144:8192], io["wpack"][:, 6144:8192])

        with tc.tile_pool(name="qk0", bufs=1, space="PSUM") as qk, \
             tc.tile_pool(name="v0", bufs=2, space="PSUM") as v0:
            proj_chunk_emajor(0, xt0, qk, v0)

        psMix = ctx.enter_context(tc.tile_pool(name="psMix", bufs=3, space="PSUM"))
        psS = ctx.enter_context(tc.tile_pool(name="psS", bufs=2, space="PSUM"))
        tpool = ctx.enter_context(tc.tile_pool(name="tpool", bufs=1, space="PSUM"))
        psW = psMix
        uA = ctx.enter_context(tc.tile_pool(name="uA", bufs=1, space="PSUM"))
        uB = ctx.enter_context(tc.tile_pool(name="uB", bufs=1, space="PSUM"))
        # u slots: 3 rotating qb slots x 2 heads per bank (cols 0:390);
        # transpose slots live in the banks' spare cols 448:512 as bf16.
        # start=True must never touch these banks: memset + start=False.
        ubankA = uA.tile([128, 512], F32, tag="uA")
        ubankB = uB.tile([128, 512], F32, tag="uB")
        _ub = [ubankA, ubankB]
        g.uslot = lambda qb, h: _ub[h // 2][
            :, 130 * (qb % 3) + 65 * (h % 2) : 130 * (qb % 3) + 65 * (h % 2) + 65
        ]
        tbank = tpool.tile([128, 256], BF16, tag="tbank")
        g.tslot = lambda s: tbank[:, 128 * (s % 2) : 128 * (s % 2) + 128]

        attn_pass(bg=chunk_steps(1, xt1))
        drain_wo(64, pools=[(psMix, "p")])

        if "dbg" in io:
            nc.sync.dma_start(io["dbg"][:, 0 : 2 * S], g.qT[:])
            nc.sync.dma_start(io["dbg"][:, 2 * S : 4 * S], g.kT[:])
            nc.sync.dma_start(io["dbg"][:, 4 * S : 4 * S + NB * 256], g.aoTM[:])
            if not g.no_tp:
                nc.sync.dma_start(io["dbg"][:, 6 * S : 8 * S], g.aoTd[:])
            nc.sync.dma_start(io["dbg"][:, 8 * S : 8 * S + NB * 260], g.vA[:])


def build_program(dbg=False):
    nc = bacc.Bacc("TRN2", target_bir_lowering=False, debug=False, num_devices=N_CORES)
    io = {}

    def inp(name, shape):
        io[name] = nc.dram_tensor(name, shape, BF16, kind="ExternalInput").ap()

    inp("xT", [8, 128, S])
    inp("wpack", [128, 8192])
    inp("mpack", [128, 384])
    io["outT"] = nc.dram_tensor("outT", [8, 128, S], BF16, kind="ExternalOutput").ap()
    if dbg:
        io["dbg"] = nc.dram_tensor("dbg", [128, 8 * S + NB * 260], BF16,
                                   kind="ExternalOutput").ap()

    with tile.TileContext(nc) as tc:
        _emit(tc, io)
    nc.compile()
    return nc


def _host_inputs(x, Wq, Wk, Wv, Wo):
    """Per-core input maps. Core index = 2*hg + b."""
    r = np.arange(128)[:, None]
    c = np.arange(128)[None, :]
    m_left = (r <= c).astype(np.float32)
    m_right = (r >= c).astype(np.float32)
    mpack = np.concatenate(
        [m_left, m_right, np.eye(128, dtype=np.float32)], axis=1
    ).astype(bfloat16)

    scale = 1.0 / np.sqrt(D)
    xTb = []
    for b in range(B):
        xf = np.ascontiguousarray(x[b].T).astype(bfloat16)   # [1024, 2048]
        xTb.append(xf.reshape(8, 128, S))

    in_maps = []
    for core in range(N_CORES):
        hg, b = core // 2, core % 2
        rows = slice(256 * hg, 256 * hg + 256)
        wq = np.ascontiguousarray((Wq[rows, :] * scale).T)   # [1024 e, 256 d']
        wk = np.ascontiguousarray(Wk[rows, :].T)
        wv = np.ascontiguousarray(Wv[rows, :].T)
        wqc = wq.reshape(8, 128, 256)
        wkc = wk.reshape(8, 128, 256)
        wvc = wv.reshape(8, 128, 256)
        wpk = np.zeros((128, 8192), dtype=np.float32)
        for e in range(8):
            wpk[:, 0 + 256 * e : 256 * e + 256] = wqc[e]
            wpk[:, 2048 + 256 * e : 2304 + 256 * e] = wkc[e]
            wpk[:, 4096 + 256 * e : 4352 + 256 * e] = wvc[e]
        for s in range(2):
            wos = Wo[:, 256 * hg + 128 * s : 256 * hg + 128 * s + 128].T  # [128, 1024]
            for m in range(8):
                wpk[:, 6144 + 1024 * s + 128 * m : 6144 + 1024 * s + 128 * m + 128] = \
                    wos[:, 128 * m : 128 * m + 128]
        in_maps.append(
            {"xT": xTb[b], "wpack": wpk.astype(bfloat16), "mpack": mpack}
        )
    return in_maps


_NC_CACHE = None


def kernel(x, Wq, Wk, Wv, Wo, bo):
    global _NC_CACHE
    x = np.asarray(x, dtype=np.float32)
    Wq = np.asarray(Wq, dtype=np.float32)
    Wk = np.asarray(Wk, dtype=np.float32)
    Wv = np.asarray(Wv, dtype=np.float32)
    Wo = np.asarray(Wo, dtype=np.float32)
    bo = np.asarray(bo, dtype=np.float32)

    if _NC_CACHE is None:
        _NC_CACHE = build_program()
    nc = _NC_CACHE

    in_maps = _host_inputs(x, Wq, Wk, Wv, Wo)
    res = run_bass_kernel_spmd(nc, in_maps, core_ids=list(range(N_CORES)))

    out = np.zeros((B, S, E), dtype=np.float32)
    for core in range(N_CORES):
        hg, b = core // 2, core % 2
        out[b] += res.results[core]["outT"].astype(np.float32).reshape(E, S).T
    return np.ascontiguousarray(out + bo[None, None, :])
